# revision 19
# baseline (speedup 1.0000x reference)
"""MoE feed-forward (E=8 experts, top-2) for one TRN2 chip (8 NeuronCores).

Strategy: expert-parallel. Host computes the (tiny) router matmul + softmax
+ top-2 in numpy, gathers each expert's routed tokens, pads to a fixed
capacity C, and ships per-expert weights + gathered tokens to one core each.
Each core runs an identical Bass/Tile FFN program in bf16:

    GT = Wg^T @ X   (transposed-activation layout: [I, C] tiles)
    UT = Wu^T @ X
    AT = silu(GT) * UT          (bf16, SBUF-resident)
    YT = Wd^T_col-tiles @ AT    -> [H, C] bf16 out

The PE stream (~166us of bf16 matmul at C~1078) is the wall; the program
is organized to keep every non-PE cost off the critical path:
  - token chunks are [512, C-640, 128]: all matmuls are >=128 cols wide
    (narrow matmuls hit a ~34ns/instr issue floor), and the LAST chunk is
    128 wide so the final cast + output DMA tail is ~3x shorter than an
    even split;
  - phase A (gate/up+silu+mul) runs chunk-major (all 16 i-tiles for chunk
    0, then chunk 1, ...), phase B (down-proj) h-tile passes interleave
    between phase A passes (A-k0, A-k1, B-k0, A-k2, B-k1, B-k2) so the
    down-projection never waits on the tail of phase A and output DMA is
    spread over the second half of the program;
  - input DMAs are all issued up front from BOTH HWDGE queues in
    consumption order: ACT carries i-tile 0's gate weights split per-c
    (first matmul waits on 32KB, not 256KB) plus the first few i-tiles'
    gate/up weights; SP carries x chunk-0 pieces then the rest;
  - the PE clock ramps 0.65->1.2->2.4GHz and needs ~3us of GAPLESS
    execution to reach full speed (any stall resets the ramp), so
    dependency-free junk matmuls bridge the fixed ~6.5us engine-start
    latency and the early DMA-paced stretch of the first i-tile sweep;
  - per (i-tile, chunk) only one g/u PSUM pair is live and is consumed
    (silu+mul) while the next i-tile's matmuls run — no PSUM-slot stalls;
  - y is written out in bf16 (error contribution ~0.2% of an output that
    has ~8x that from the bf16 matmuls).

The host applies the top-2 combine weights and scatters rows back into the
full [B, S, H] output.
"""

import numpy as np
import ml_dtypes

H = 1024
I = 2048
E = 8
TOPK = 2
P = 128

_PROGRAM_CACHE = {}
LAST_RESULT = None  # BassKernelResults of the most recent device run


def _chunk_widths(C):
    """Split C columns into PSUM-sized chunks (<=512), all >=128 wide,
    with a 128-wide tail chunk to shrink the output flush."""
    assert C % 2 == 0 and C >= 256
    if C <= 512:
        return [C - 128, 128] if C > 256 else [C]
    body = C - 128
    n = -(-body // 512)  # chunks of <=512 for the body
    base = body // n
    widths = [base + (1 if i < body % n else 0) for i in range(n)]
    assert all(128 <= w <= 512 for w in widths)
    return widths + [128]


def _build_program(C):
    from contextlib import ExitStack

    import concourse.mybir as mybir
    import concourse.tile as tile
    from concourse import bacc

    f32 = mybir.dt.float32
    bf16 = mybir.dt.bfloat16
    Silu = mybir.ActivationFunctionType.Silu

    n_h = H // P   # 8 contraction chunks over hidden dim
    n_i = I // P   # 16 tiles over intermediate dim
    widths = _chunk_widths(C)
    starts = [sum(widths[:i]) for i in range(len(widths))]
    nk = len(widths)

    nc = bacc.Bacc("TRN2", enable_partition_id=False)
    xT = nc.dram_tensor("xT", [H, C], bf16, kind="ExternalInput")
    # gate/up are host-prearranged to [p, i_tile, c, i_within] so each
    # per-i-tile DMA reads 2KB-contiguous lines per partition.
    wg = nc.dram_tensor("wg", [P, I // P, H // P, P], bf16, kind="ExternalInput")
    wu = nc.dram_tensor("wu", [P, I // P, H // P, P], bf16, kind="ExternalInput")
    wd = nc.dram_tensor("wd", [I, H], bf16, kind="ExternalInput")
    yT = nc.dram_tensor("yT", [H, C], bf16, kind="ExternalOutput")

    with tile.TileContext(nc) as tc:
        with ExitStack() as ctx:
            wpool = ctx.enter_context(tc.tile_pool(name="weights", bufs=1))
            atpool = ctx.enter_context(tc.tile_pool(name="atp", bufs=1))
            spool = ctx.enter_context(tc.tile_pool(name="stmp", bufs=4))
            ypool = ctx.enter_context(tc.tile_pool(name="yst", bufs=4))
            pspool = ctx.enter_context(
                tc.tile_pool(name="ps", bufs=8, space="PSUM")
            )

            warm_src = wpool.tile([P, P], bf16, name="warm_src")
            nc.vector.memset(warm_src, 0.0)
            warm_ps = pspool.tile([P, 512], f32, tag="ps", name="warm_ps")
            # Dummy activation: forces the 1.3us Silu ACT_TABLE_LOAD to
            # happen during the DMA-paced warmup era instead of blocking
            # the first real silu (whose delay would back up PSUM and
            # stall the PE).
            warm_act = spool.tile([P, 1], f32, tag="stmp", name="warm_act")
            nc.scalar.activation(warm_act, warm_src[:, 0:1], Silu)

            def junk_mms(n):
                # Dependency-free matmuls that keep the PE busy while real
                # operands are still in flight. Any PE-idle window resets
                # the clock-ramp monitor (back to 1.2GHz), so junk must
                # precisely bridge every DMA-paced gap at the head.
                for _ in range(n):
                    nc.tensor.matmul(
                        warm_ps[:, 0:P], warm_src, warm_src,
                        start=True, stop=True,
                    )

            junk_mms(12)

            x_s = wpool.tile([P, n_h, C], bf16, name="x_s")
            wg_s = wpool.tile([P, n_i, n_h, P], bf16, name="wg_s")
            wu_s = wpool.tile([P, n_i, n_h, P], bf16, name="wu_s")
            wd_s = wpool.tile([P, n_i, H], bf16, name="wd_s")
            at_s = atpool.tile([P, n_i, C], bf16, name="at_s")

            # Input DMAs, all issued up front, in consumption order.
            # ACT queue: ONLY gate/up i-tiles 0 (wg0 split per-c so the
            # first matmul waits on one 32KB piece) and 1 — few enough
            # issues that the scalar engine frees up to run the phase-A
            # silus (a backed-up scalar stream delays silu, fills all 8
            # PSUM banks, and stalls the PE; gpsimd is no alternative —
            # its SWDGE descriptor generation is ~0.5us per DMA).
            # SP queue: x chunk-0 pieces, gate/up i-tiles 2..15,
            # remaining x chunks, then wd.
            # A single dma_start lands on ONE of the 16 HW DMA engines
            # (~22GB/s each); aggregate bandwidth needs many outstanding
            # transfers.  The first sweep's operands are therefore split
            # across partition ranges so several engines carry each piece
            # in parallel — otherwise x chunk-0 trickles in at one-engine
            # speed and the PE idles ~10us at the head.
            # Early input is line-rate-bound per queue, so spread it over
            # FOUR queues: ACT carries i-tile 0/1 gate/up (wg0/wu0 split
            # per-c so the first matmuls wait on 32KB pieces), SP and
            # Vector alternate the x h-blocks (full width: 2156B lines,
            # and chunks 1/2 ride along for free), then SP/Vector split
            # the remaining gate/up i-tiles.  Vector's own first multiply
            # isn't until ~19us, so its queue is free at the head.
            def dma_x(c):
                return nc_eng_x[c].dma_start(
                    out=x_s[:, c, :], in_=xT[c * P:(c + 1) * P, :]
                )

            nc_eng_x = [nc.sync, nc.scalar, nc.sync, nc.scalar,
                        nc.sync, nc.sync, nc.sync, nc.sync]
            for c in range(n_h):
                nc.scalar.dma_start(
                    out=wg_s[:, 0, c:c + 1, :], in_=wg[:, 0, c:c + 1, :]
                )
            dma_x(0)
            dma_x(1)
            dma_x(2)
            for c in range(n_h):
                nc.scalar.dma_start(
                    out=wu_s[:, 0, c:c + 1, :], in_=wu[:, 0, c:c + 1, :]
                )
            dma_x(4)
            dma_x(3)
            dma_x(5)
            nc.scalar.dma_start(out=wg_s[:, 1, :, :], in_=wg[:, 1, :, :])
            dma_x(6)
            nc.scalar.dma_start(out=wu_s[:, 1, :, :], in_=wu[:, 1, :, :])
            dma_x(7)
            for it in range(2, n_i):
                nc.sync.dma_start(out=wg_s[:, it, :, :], in_=wg[:, it, :, :])
                nc.sync.dma_start(out=wu_s[:, it, :, :], in_=wu[:, it, :, :])
            for it in range(n_i):
                nc.sync.dma_start(
                    out=wd_s[:, it, :], in_=wd[it * P:(it + 1) * P, :]
                )

            def a_pass(k):
                # AT[:, it, chunk k] = silu(Wg^T X) * (Wu^T X) for all it
                c0, w = starts[k], widths[k]
                for it in range(n_i):
                    g_ps = pspool.tile([P, w], f32, tag="ps", name=f"g_{it}_{k}")
                    u_ps = pspool.tile([P, w], f32, tag="ps", name=f"u_{it}_{k}")
                    # g-sweep before u-sweep: the silu on g_ps starts while
                    # the u-sweep still runs. The very first sweep is paced
                    # by the trickling x pieces, so junk matmuls fill those
                    # gaps to keep the clock-ramp monitor busy.
                    for c in range(n_h):
                        st, sp = (c == 0), (c == n_h - 1)
                        nc.tensor.matmul(
                            g_ps, wg_s[:, it, c, :],
                            x_s[:, c, c0:c0 + w], start=st, stop=sp,
                        )
                        if k == 0 and it == 0 and c < n_h - 1:
                            # taper matched to the early x-piece cadence;
                            # slight overfill is cheap (56ns/junk) while a
                            # gap costs 1-3us of clock re-ramp
                            junk_mms((6, 10, 10, 4, 12, 12, 12)[c])
                    if k == 0 and it == 0:
                        junk_mms(4)
                    for c in range(n_h):
                        st, sp = (c == 0), (c == n_h - 1)
                        nc.tensor.matmul(
                            u_ps, wu_s[:, it, c, :],
                            x_s[:, c, c0:c0 + w], start=st, stop=sp,
                        )
                    stile = spool.tile([P, w], f32, tag="stmp", name=f"s_{it}_{k}")
                    nc.scalar.activation(stile, g_ps, Silu)
                    nc.vector.tensor_mul(
                        at_s[:, it, c0:c0 + w], stile, u_ps
                    )

            def b_pass(k, last=False):
                # YT[ht, chunk k] = sum_i Wd[i, ht]^T AT[i, chunk k]
                c0, w = starts[k], widths[k]
                for ht in range(n_h):
                    y_ps = pspool.tile([P, w], f32, tag="ps", name=f"y_{ht}_{k}")
                    for it in range(n_i):
                        st, sp = (it == 0), (it == n_i - 1)
                        nc.tensor.matmul(
                            y_ps, wd_s[:, it, ht * P:(ht + 1) * P],
                            at_s[:, it, c0:c0 + w], start=st, stop=sp,
                        )
                    yt = ypool.tile([P, w], bf16, tag="yst", name=f"yo_{ht}_{k}")
                    nc.vector.tensor_copy(yt, y_ps)
                    if not (last and ht == n_h - 1):
                        eng = nc.sync if ht % 2 == 0 else nc.scalar
                        eng.dma_start(
                            out=yT[ht * P:(ht + 1) * P, c0:c0 + w], in_=yt,
                        )
                    else:
                        # Final flush: the last chunk is only 128 wide, so
                        # split across both queues to shrink the tail.
                        q = P // 2
                        for r in range(2):
                            eng = nc.sync if r == 0 else nc.scalar
                            eng.dma_start(
                                out=yT[ht * P + r * q:ht * P + (r + 1) * q,
                                       c0:c0 + w],
                                in_=yt[r * q:(r + 1) * q, :],
                            )

            # Interleave: down-projection passes slot between phase A
            # passes; the last pass is the 128-wide chunk's down-proj.
            order = []
            for k in range(nk):
                if k >= 2:
                    order.append(("b", k - 2))
                order.append(("a", k))
            order.append(("b", nk - 2))
            order.append(("b", nk - 1))
            for ph, k in order:
                if ph == "a":
                    a_pass(k)
                else:
                    b_pass(k, last=(k == nk - 1))

    nc.compile()
    return nc


def kernel(x, gate_w, wg, wu, wd):
    global LAST_RESULT
    x = np.asarray(x, dtype=np.float32)
    gate_w = np.asarray(gate_w, dtype=np.float32)
    wg = np.asarray(wg, dtype=np.float32)
    wu = np.asarray(wu, dtype=np.float32)
    wd = np.asarray(wd, dtype=np.float32)

    B, S, Hh = x.shape
    T = B * S
    xf = np.ascontiguousarray(x.reshape(T, Hh))

    # Router (tiny): logits -> softmax -> top-2, matching jax.lax.top_k
    # tie-order (stable sort prefers the lower expert index).
    logits = xf @ gate_w.T
    logits -= logits.max(axis=-1, keepdims=True)
    np.exp(logits, out=logits)
    probs = logits / logits.sum(axis=-1, keepdims=True)
    order = np.argsort(-probs, axis=1, kind="stable")[:, :TOPK]

    onehot = np.zeros((T, E), dtype=bool)
    onehot[np.arange(T)[:, None], order] = True
    tok_lists = [np.nonzero(onehot[:, e])[0] for e in range(E)]
    maxc = max(max(len(t) for t in tok_lists), 256)
    C = maxc + (maxc % 2)  # round up to even
    # PSUM chunking and SBUF residency cap C; the expected per-expert load
    # is T*TOPK/E = 1024, so this is ample margin.
    assert C <= 1344, f"expert load too imbalanced for this kernel: {maxc}"

    nc = _PROGRAM_CACHE.get(C)
    if nc is None:
        nc = _build_program(C)
        _PROGRAM_CACHE[C] = nc

    bf = ml_dtypes.bfloat16
    xf_bf = xf.astype(bf)

    def _gu_layout(w):  # [H, I] -> [p, i_tile, c, j]
        return np.ascontiguousarray(
            w.reshape(H // P, P, I // P, P).transpose(1, 2, 0, 3)
        )

    in_maps = []
    for e in range(E):
        idx = tok_lists[e]
        xe = np.zeros((C, Hh), dtype=bf)
        xe[: len(idx)] = xf_bf[idx]
        in_maps.append(
            {
                "xT": np.ascontiguousarray(xe.T),
                "wg": _gu_layout(wg[e].astype(bf)),
                "wu": _gu_layout(wu[e].astype(bf)),
                "wd": wd[e].astype(bf),
            }
        )

    from concourse.bass_utils import run_bass_kernel_spmd

    res = run_bass_kernel_spmd(nc, in_maps, core_ids=list(range(E)))
    LAST_RESULT = res

    out = np.zeros((T, Hh), dtype=np.float32)
    for e in range(E):
        idx = tok_lists[e]
        ye = np.asarray(res.results[e]["yT"]).T[: len(idx)]
        out[idx] += probs[idx, e][:, None] * ye.astype(np.float32)
    return out.reshape(B, S, Hh)


# revision 27
# speedup vs baseline: 1.0586x; 1.0586x over previous
"""Original baseline kernel (session-start state) for device-drift calibration."""

import numpy as np
import ml_dtypes

H = 1024
I = 2048
E = 8
TOPK = 2
P = 128
N_T = 3  # token chunks per core (chunk width C/3 <= 512 = one PSUM bank)

_PROGRAM_CACHE = {}
LAST_RESULT = None  # BassKernelResults of the most recent device run


def _build_program(C):
    from contextlib import ExitStack

    import concourse.mybir as mybir
    import concourse.tile as tile
    from concourse import bacc

    f32 = mybir.dt.float32
    bf16 = mybir.dt.bfloat16
    Silu = mybir.ActivationFunctionType.Silu

    n_h = H // P   # 8 contraction chunks over hidden dim
    n_i = I // P   # 16 tiles over intermediate dim
    # Asymmetric token chunks: chunk 0 stays 360 wide (the startup taper
    # is tuned to its ~90KB x-piece cadence), chunk 1 takes a full PSUM
    # bank (512), and the remainder lands in a small LAST chunk so the
    # final cast + output-DMA tail after the last matmul is short.
    k1 = min(512, C - 360 - 128)
    widths = [360, k1, C - 360 - k1]
    assert all(82 <= w <= 512 for w in widths), widths
    starts = [0, 360, 360 + k1]
    NTmax = max(widths)

    nc = bacc.Bacc("TRN2", enable_partition_id=False)
    xT = nc.dram_tensor("xT", [H, C], bf16, kind="ExternalInput")
    wg = nc.dram_tensor("wg", [P, I // P, H // P, P], bf16, kind="ExternalInput")
    wu = nc.dram_tensor("wu", [P, I // P, H // P, P], bf16, kind="ExternalInput")
    wd = nc.dram_tensor("wd", [I, H], bf16, kind="ExternalInput")
    yT = nc.dram_tensor("yT", [H, C], bf16, kind="ExternalOutput")

    with tile.TileContext(nc) as tc:
        with ExitStack() as ctx:
            wpool = ctx.enter_context(tc.tile_pool(name="weights", bufs=1))
            atpool = ctx.enter_context(tc.tile_pool(name="atp", bufs=1))
            spool = ctx.enter_context(tc.tile_pool(name="stmp", bufs=4))
            ypool = ctx.enter_context(tc.tile_pool(name="yst", bufs=4))
            pspool = ctx.enter_context(
                tc.tile_pool(name="ps", bufs=8, space="PSUM")
            )

            warm_src = wpool.tile([P, P], bf16, name="warm_src")
            nc.vector.memset(warm_src, 0.0)
            warm_ps = pspool.tile([P, NTmax], f32, tag="ps", name="warm_ps")
            # Dummy activation: pulls the ~1.3us Silu ACT_TABLE_LOAD into
            # the DMA-paced warmup era so the first real silu runs at full
            # speed (a late silu backs up PSUM and stalls the PE).
            warm_act = spool.tile([P, 1], f32, tag="stmp", name="warm_act")
            nc.scalar.activation(warm_act, warm_src[:, 0:1], Silu)

            def junk_mms(n):
                for _ in range(n):
                    nc.tensor.matmul(
                        warm_ps[:, 0:P], warm_src, warm_src,
                        start=True, stop=True,
                    )

            junk_mms(26)

            x_s = wpool.tile([P, n_h, C], bf16, name="x_s")
            wg_s = wpool.tile([P, n_i, n_h, P], bf16, name="wg_s")
            wu_s = wpool.tile([P, n_i, n_h, P], bf16, name="wu_s")
            wd_s = wpool.tile([P, n_i, H], bf16, name="wd_s")
            at_s = atpool.tile([P, n_i, C], bf16, name="at_s")

            k0w = widths[0]
            for c in range(n_h):
                nc.scalar.dma_start(
                    out=wg_s[:, 0, c:c + 1, :], in_=wg[:, 0, c:c + 1, :]
                )
            nc.scalar.dma_start(out=wu_s[:, 0, :, :], in_=wu[:, 0, :, :])
            for c in range(n_h):
                nc.sync.dma_start(
                    out=x_s[:, c, 0:k0w], in_=xT[c * P:(c + 1) * P, 0:k0w]
                )
            nc.sync.dma_start(out=wg_s[:, 1, :, :], in_=wg[:, 1, :, :])
            nc.sync.dma_start(out=wu_s[:, 1, :, :], in_=wu[:, 1, :, :])
            for it in range(2, n_i):
                nc.sync.dma_start(out=wg_s[:, it, :, :], in_=wg[:, it, :, :])
                nc.sync.dma_start(out=wu_s[:, it, :, :], in_=wu[:, it, :, :])
            for c in range(n_h):
                nc.sync.dma_start(
                    out=x_s[:, c, k0w:C], in_=xT[c * P:(c + 1) * P, k0w:C]
                )
            for it in range(n_i):
                nc.sync.dma_start(
                    out=wd_s[:, it, :], in_=wd[it * P:(it + 1) * P, :]
                )

            def a_pass(k):
                c0, w = starts[k], widths[k]
                for it in range(n_i):
                    g_ps = pspool.tile([P, w], f32, tag="ps", name=f"g_{it}_{k}")
                    u_ps = pspool.tile([P, w], f32, tag="ps", name=f"u_{it}_{k}")
                    for c in range(n_h):
                        st, sp = (c == 0), (c == n_h - 1)
                        nc.tensor.matmul(
                            g_ps, wg_s[:, it, c, :],
                            x_s[:, c, c0:c0 + w], start=st, stop=sp,
                        )
                        if k == 0 and it == 0 and c < n_h - 1:
                            junk_mms((10, 10, 6, 3, 2, 1, 0)[c])
                    for c in range(n_h):
                        st, sp = (c == 0), (c == n_h - 1)
                        nc.tensor.matmul(
                            u_ps, wu_s[:, it, c, :],
                            x_s[:, c, c0:c0 + w], start=st, stop=sp,
                        )
                    stile = spool.tile([P, w], f32, tag="stmp", name=f"s_{it}_{k}")
                    nc.scalar.activation(stile, g_ps, Silu)
                    nc.vector.tensor_mul(
                        at_s[:, it, c0:c0 + w], stile, u_ps
                    )

            def b_pass(k, last=False):
                c0, w = starts[k], widths[k]
                for ht in range(n_h):
                    y_ps = pspool.tile([P, w], f32, tag="ps", name=f"y_{ht}_{k}")
                    for it in range(n_i):
                        st, sp = (it == 0), (it == n_i - 1)
                        nc.tensor.matmul(
                            y_ps, wd_s[:, it, ht * P:(ht + 1) * P],
                            at_s[:, it, c0:c0 + w], start=st, stop=sp,
                        )
                    yt = ypool.tile([P, w], bf16, tag="yst", name=f"yo_{ht}_{k}")
                    nc.vector.tensor_copy(yt, y_ps)
                    if not (last and ht == n_h - 1):
                        nc.sync.dma_start(
                            out=yT[ht * P:(ht + 1) * P, c0:c0 + w],
                            in_=yt,
                        )
                    else:
                        q = P // 4
                        for r in range(4):
                            eng = nc.sync if r % 2 == 0 else nc.scalar
                            eng.dma_start(
                                out=yT[ht * P + r * q:ht * P + (r + 1) * q,
                                       c0:c0 + w],
                                in_=yt[r * q:(r + 1) * q, :],
                            )

            a_pass(0)
            a_pass(1)
            b_pass(0)
            a_pass(2)
            b_pass(1)
            b_pass(2, last=True)

    nc.compile()
    return nc


def kernel(x, gate_w, wg, wu, wd):
    global LAST_RESULT
    x = np.asarray(x, dtype=np.float32)
    gate_w = np.asarray(gate_w, dtype=np.float32)
    wg = np.asarray(wg, dtype=np.float32)
    wu = np.asarray(wu, dtype=np.float32)
    wd = np.asarray(wd, dtype=np.float32)

    B, S, Hh = x.shape
    T = B * S
    xf = np.ascontiguousarray(x.reshape(T, Hh))

    logits = xf @ gate_w.T
    logits -= logits.max(axis=-1, keepdims=True)
    np.exp(logits, out=logits)
    probs = logits / logits.sum(axis=-1, keepdims=True)
    order = np.argsort(-probs, axis=1, kind="stable")[:, :TOPK]

    onehot = np.zeros((T, E), dtype=bool)
    onehot[np.arange(T)[:, None], order] = True
    tok_lists = [np.nonzero(onehot[:, e])[0] for e in range(E)]
    maxc = max(max(len(t) for t in tok_lists), 600)
    C = maxc + (maxc % 2)  # round up to even
    assert C <= 1344, f"expert load too imbalanced for this kernel: {maxc}"

    nc = _PROGRAM_CACHE.get(C)
    if nc is None:
        nc = _build_program(C)
        _PROGRAM_CACHE[C] = nc

    bf = ml_dtypes.bfloat16
    xf_bf = xf.astype(bf)

    def _gu_layout(w):
        return np.ascontiguousarray(
            w.reshape(H // P, P, I // P, P).transpose(1, 2, 0, 3)
        )

    in_maps = []
    for e in range(E):
        idx = tok_lists[e]
        xe = np.zeros((C, Hh), dtype=bf)
        xe[: len(idx)] = xf_bf[idx]
        in_maps.append(
            {
                "xT": np.ascontiguousarray(xe.T),
                "wg": _gu_layout(wg[e].astype(bf)),
                "wu": _gu_layout(wu[e].astype(bf)),
                "wd": wd[e].astype(bf),
            }
        )

    from concourse.bass_utils import run_bass_kernel_spmd

    res = run_bass_kernel_spmd(nc, in_maps, core_ids=list(range(E)))
    LAST_RESULT = res

    out = np.zeros((T, Hh), dtype=np.float32)
    for e in range(E):
        idx = tok_lists[e]
        ye = np.asarray(res.results[e]["yT"]).T[: len(idx)]
        out[idx] += probs[idx, e][:, None] * ye.astype(np.float32)
    return out.reshape(B, S, Hh)


# revision 29
# speedup vs baseline: 1.0709x; 1.0116x over previous
"""Original baseline kernel (session-start state) for device-drift calibration."""

import numpy as np
import ml_dtypes

H = 1024
I = 2048
E = 8
TOPK = 2
P = 128
N_T = 3  # token chunks per core (chunk width C/3 <= 512 = one PSUM bank)

_PROGRAM_CACHE = {}
LAST_RESULT = None  # BassKernelResults of the most recent device run


def _build_program(C):
    from contextlib import ExitStack

    import concourse.mybir as mybir
    import concourse.tile as tile
    from concourse import bacc

    f32 = mybir.dt.float32
    bf16 = mybir.dt.bfloat16
    Silu = mybir.ActivationFunctionType.Silu

    n_h = H // P   # 8 contraction chunks over hidden dim
    n_i = I // P   # 16 tiles over intermediate dim
    # Asymmetric token chunks: chunk 0 stays 360 wide (the startup taper
    # is tuned to its ~90KB x-piece cadence), chunk 1 takes a full PSUM
    # bank (512), and the remainder lands in a small LAST chunk so the
    # final cast + output-DMA tail after the last matmul is short.
    k1 = min(512, C - 360 - 128)
    widths = [360, k1, C - 360 - k1]
    assert all(82 <= w <= 512 for w in widths), widths
    starts = [0, 360, 360 + k1]
    NTmax = max(widths)

    nc = bacc.Bacc("TRN2", enable_partition_id=False)
    xT = nc.dram_tensor("xT", [H, C], bf16, kind="ExternalInput")
    wg = nc.dram_tensor("wg", [P, I // P, H // P, P], bf16, kind="ExternalInput")
    wu = nc.dram_tensor("wu", [P, I // P, H // P, P], bf16, kind="ExternalInput")
    wd = nc.dram_tensor("wd", [I, H], bf16, kind="ExternalInput")
    yT = nc.dram_tensor("yT", [H, C], bf16, kind="ExternalOutput")

    with tile.TileContext(nc) as tc:
        with ExitStack() as ctx:
            wpool = ctx.enter_context(tc.tile_pool(name="weights", bufs=1))
            atpool = ctx.enter_context(tc.tile_pool(name="atp", bufs=1))
            spool = ctx.enter_context(tc.tile_pool(name="stmp", bufs=4))
            ypool = ctx.enter_context(tc.tile_pool(name="yst", bufs=4))
            pspool = ctx.enter_context(
                tc.tile_pool(name="ps", bufs=8, space="PSUM")
            )

            warm_src = wpool.tile([P, P], bf16, name="warm_src")
            nc.vector.memset(warm_src, 0.0)
            warm_ps = pspool.tile([P, NTmax], f32, tag="ps", name="warm_ps")

            def junk_mms(n):
                for _ in range(n):
                    nc.tensor.matmul(
                        warm_ps[:, 0:P], warm_src, warm_src,
                        start=True, stop=True,
                    )

            junk_mms(26)

            x_s = wpool.tile([P, n_h, C], bf16, name="x_s")
            wg_s = wpool.tile([P, n_i, n_h, P], bf16, name="wg_s")
            wu_s = wpool.tile([P, n_i, n_h, P], bf16, name="wu_s")
            wd_s = wpool.tile([P, n_i, H], bf16, name="wd_s")
            at_s = atpool.tile([P, n_i, C], bf16, name="at_s")

            k0w = widths[0]
            for c in range(n_h):
                nc.scalar.dma_start(
                    out=wg_s[:, 0, c:c + 1, :], in_=wg[:, 0, c:c + 1, :]
                )
            nc.scalar.dma_start(out=wu_s[:, 0, :, :], in_=wu[:, 0, :, :])
            for c in range(n_h):
                nc.sync.dma_start(
                    out=x_s[:, c, 0:k0w], in_=xT[c * P:(c + 1) * P, 0:k0w]
                )
            nc.sync.dma_start(out=wg_s[:, 1, :, :], in_=wg[:, 1, :, :])
            nc.sync.dma_start(out=wu_s[:, 1, :, :], in_=wu[:, 1, :, :])
            for it in range(2, n_i):
                nc.sync.dma_start(out=wg_s[:, it, :, :], in_=wg[:, it, :, :])
                nc.sync.dma_start(out=wu_s[:, it, :, :], in_=wu[:, it, :, :])
            for c in range(n_h):
                nc.sync.dma_start(
                    out=x_s[:, c, k0w:C], in_=xT[c * P:(c + 1) * P, k0w:C]
                )
            for it in range(n_i):
                nc.sync.dma_start(
                    out=wd_s[:, it, :], in_=wd[it * P:(it + 1) * P, :]
                )

            # Dummy activation AFTER the scalar queue's DMA issues: pulls
            # the ~1.3us Silu ACT_TABLE_LOAD into the DMA-paced warmup era
            # (so the first real silu doesn't stall PSUM) without delaying
            # the wg0-piece DMAs the first matmuls wait on.
            warm_act = spool.tile([P, 1], f32, tag="stmp", name="warm_act")
            nc.scalar.activation(warm_act, warm_src[:, 0:1], Silu)

            def a_pass(k):
                c0, w = starts[k], widths[k]
                for it in range(n_i):
                    g_ps = pspool.tile([P, w], f32, tag="ps", name=f"g_{it}_{k}")
                    u_ps = pspool.tile([P, w], f32, tag="ps", name=f"u_{it}_{k}")
                    for c in range(n_h):
                        st, sp = (c == 0), (c == n_h - 1)
                        nc.tensor.matmul(
                            g_ps, wg_s[:, it, c, :],
                            x_s[:, c, c0:c0 + w], start=st, stop=sp,
                        )
                        if k == 0 and it == 0 and c < n_h - 1:
                            junk_mms((10, 10, 6, 3, 2, 1, 0)[c])
                    for c in range(n_h):
                        st, sp = (c == 0), (c == n_h - 1)
                        nc.tensor.matmul(
                            u_ps, wu_s[:, it, c, :],
                            x_s[:, c, c0:c0 + w], start=st, stop=sp,
                        )
                    stile = spool.tile([P, w], f32, tag="stmp", name=f"s_{it}_{k}")
                    nc.scalar.activation(stile, g_ps, Silu)
                    nc.vector.tensor_mul(
                        at_s[:, it, c0:c0 + w], stile, u_ps
                    )

            def b_pass(k, last=False):
                c0, w = starts[k], widths[k]
                for ht in range(n_h):
                    y_ps = pspool.tile([P, w], f32, tag="ps", name=f"y_{ht}_{k}")
                    for it in range(n_i):
                        st, sp = (it == 0), (it == n_i - 1)
                        nc.tensor.matmul(
                            y_ps, wd_s[:, it, ht * P:(ht + 1) * P],
                            at_s[:, it, c0:c0 + w], start=st, stop=sp,
                        )
                    yt = ypool.tile([P, w], bf16, tag="yst", name=f"yo_{ht}_{k}")
                    nc.vector.tensor_copy(yt, y_ps)
                    if not (last and ht == n_h - 1):
                        nc.sync.dma_start(
                            out=yT[ht * P:(ht + 1) * P, c0:c0 + w],
                            in_=yt,
                        )
                    else:
                        q = P // 4
                        for r in range(4):
                            eng = nc.sync if r % 2 == 0 else nc.scalar
                            eng.dma_start(
                                out=yT[ht * P + r * q:ht * P + (r + 1) * q,
                                       c0:c0 + w],
                                in_=yt[r * q:(r + 1) * q, :],
                            )

            a_pass(0)
            a_pass(1)
            b_pass(0)
            a_pass(2)
            b_pass(1)
            b_pass(2, last=True)

    nc.compile()
    return nc


def kernel(x, gate_w, wg, wu, wd):
    global LAST_RESULT
    x = np.asarray(x, dtype=np.float32)
    gate_w = np.asarray(gate_w, dtype=np.float32)
    wg = np.asarray(wg, dtype=np.float32)
    wu = np.asarray(wu, dtype=np.float32)
    wd = np.asarray(wd, dtype=np.float32)

    B, S, Hh = x.shape
    T = B * S
    xf = np.ascontiguousarray(x.reshape(T, Hh))

    logits = xf @ gate_w.T
    logits -= logits.max(axis=-1, keepdims=True)
    np.exp(logits, out=logits)
    probs = logits / logits.sum(axis=-1, keepdims=True)
    order = np.argsort(-probs, axis=1, kind="stable")[:, :TOPK]

    onehot = np.zeros((T, E), dtype=bool)
    onehot[np.arange(T)[:, None], order] = True
    tok_lists = [np.nonzero(onehot[:, e])[0] for e in range(E)]
    maxc = max(max(len(t) for t in tok_lists), 600)
    C = maxc + (maxc % 2)  # round up to even
    assert C <= 1344, f"expert load too imbalanced for this kernel: {maxc}"

    nc = _PROGRAM_CACHE.get(C)
    if nc is None:
        nc = _build_program(C)
        _PROGRAM_CACHE[C] = nc

    bf = ml_dtypes.bfloat16
    xf_bf = xf.astype(bf)

    def _gu_layout(w):
        return np.ascontiguousarray(
            w.reshape(H // P, P, I // P, P).transpose(1, 2, 0, 3)
        )

    in_maps = []
    for e in range(E):
        idx = tok_lists[e]
        xe = np.zeros((C, Hh), dtype=bf)
        xe[: len(idx)] = xf_bf[idx]
        in_maps.append(
            {
                "xT": np.ascontiguousarray(xe.T),
                "wg": _gu_layout(wg[e].astype(bf)),
                "wu": _gu_layout(wu[e].astype(bf)),
                "wd": wd[e].astype(bf),
            }
        )

    from concourse.bass_utils import run_bass_kernel_spmd

    res = run_bass_kernel_spmd(nc, in_maps, core_ids=list(range(E)))
    LAST_RESULT = res

    out = np.zeros((T, Hh), dtype=np.float32)
    for e in range(E):
        idx = tok_lists[e]
        ye = np.asarray(res.results[e]["yT"]).T[: len(idx)]
        out[idx] += probs[idx, e][:, None] * ye.astype(np.float32)
    return out.reshape(B, S, Hh)
